# revision 32
# baseline (speedup 1.0000x reference)
"""Trainium2 Bass kernel for DigitConvolutionalModel.

Math: the 3x3 valid conv on the 28x28 image is a linear map, so it folds into
the first Linear layer:
    out = relu(x @ W_eff + b1) @ w2.T + b2
where W_eff[784, 128] = C @ w1.T and C[784, 676] is the conv-as-matrix built
from conv_w.  W_eff is built on the host (O(1) w.r.t. batch); the device does
the two batch matmuls.

Distribution: pure data parallel — batch dim of x sharded across 8 NeuronCores,
weights replicated.  Each core computes out.T [10, 8192]; the host reassembles
[65536, 10].

Layout: the contraction dim (784 features) is split 6x128 + 16.  The main
768 features ship partition-major as [128, 16, 6, 512] (partition p, batch
tile t, k-tile k, column c; feature f = k*128 + p) — 128-partition DMAs run at
~430 GB/s vs ~270 GB/s for 112-partition ones (unbalanced SDMA engine
assignment), and this is a DMA-roofline kernel.  The 16 remainder features
ship once as xrem [16, 8192] and contribute a K=16 accumulation matmul.

dtypes: x and W_eff ship as fp16 (10 mantissa bits — comparable precision to
the PE's TF32-like fp32r path at 11 bits) — halves HBM traffic and fp16
matmuls run at the full 1 cycle/row PE rate.  Accumulation is fp32 in PSUM;
the hidden activation h = relu(psum + b1) is computed on the DVE and emitted
as fp16 for the second matmul; +b2 rides the ScalarE (Identity activation).
"""

import numpy as np

import concourse.bass as bass  # noqa: F401  (bass registers mybir lowerings)
import concourse.mybir as mybir
import concourse.tile as tile
from concourse import bacc
from concourse.bass_utils import run_bass_kernel_spmd

N_CORES = 8
B = 65536
B_SH = B // N_CORES  # 8192 rows per core
D = 784              # 28*28 input features
DM = 768             # features in the main 128-partition stream
DR = D - DM          # 16 remainder features
H = 128              # hidden
OUT = 10
KT = 128             # contraction tile = full partition dim
NK = DM // KT        # 6 main K-tiles
NB = 512             # batch columns per tile (= one fp32 PSUM bank)
NT = B_SH // NB      # 16 batch tiles
G = 2                # batch tiles per x DMA (1.5 MB, 12KB/partition runs)

_CACHE = {}


def _build_nc():
    f32 = mybir.dt.float32
    f16 = mybir.dt.float16
    nc = bacc.Bacc("TRN2", target_bir_lowering=False, debug=False,
                   num_devices=N_CORES)
    # main x, partition-major: [p, t, k, c] with feature f = k*128 + p
    xtp = nc.dram_tensor("xtp", [KT, NT, NK, NB], f16,
                         kind="ExternalInput").ap()
    # remainder features 768..784: [p, batch] (base partition 0 for all rem
    # matmuls — mixing row-group tile positions reconfigures the PE array and
    # costs more than this DMA is worth)
    xrem = nc.dram_tensor("xrem", [DR, B_SH], f16, kind="ExternalInput").ap()
    weff = nc.dram_tensor("weff", [D, H], f16, kind="ExternalInput").ap()
    w2t = nc.dram_tensor("w2t", [H, OUT], f16, kind="ExternalInput").ap()
    b1c = nc.dram_tensor("b1c", [H, 1], f32, kind="ExternalInput").ap()
    b2c = nc.dram_tensor("b2c", [OUT, 1], f32, kind="ExternalInput").ap()
    out = nc.dram_tensor("out", [OUT, B_SH], f32, kind="ExternalOutput").ap()

    with tile.TileContext(nc) as tc:
        with (
            tc.tile_pool(name="wpool", bufs=1) as wpool,
            tc.tile_pool(name="xpool", bufs=6) as xpool,
            tc.tile_pool(name="hpool", bufs=4) as hpool,
            tc.tile_pool(name="opool", bufs=16) as opool,
            tc.tile_pool(name="ps1", bufs=5, space="PSUM") as ps1pool,
            tc.tile_pool(name="ps2", bufs=3, space="PSUM") as ps2pool,
        ):
            # Params + remainder features ride ring 10 (scalar) while the
            # first x group starts immediately on ring 1 (sync).  PE pre-warm:
            # dummy matmuls on a zeroed tile trip the HAM activity monitor to
            # full clock before real data arrives.
            w_sb = wpool.tile([KT, NK, H], f16)
            nc.scalar.dma_start(
                w_sb[:], weff[0:DM, :].rearrange("(k p) m -> p k m", p=KT))
            wr_sb = wpool.tile([DR, H], f16)
            nc.scalar.dma_start(wr_sb[:], weff[DM:D, :])
            w2_sb = wpool.tile([H, OUT], f16)
            nc.scalar.dma_start(w2_sb[:], w2t[:])
            b1_sb = wpool.tile([H, 1], f32)
            nc.scalar.dma_start(b1_sb[:], b1c[:])
            b2_sb = wpool.tile([OUT, 1], f32)
            nc.scalar.dma_start(b2_sb[:], b2c[:])
            xr_sb = wpool.tile([DR, B_SH], f16)
            nc.scalar.dma_start(xr_sb[:], xrem[:])

            warm_x = wpool.tile([KT, NB], f16)
            nc.vector.memset(warm_x[:], 0.0)
            warm_ps = ps1pool.tile([H, NB], f32, tag="ps1")
            for _ in range(20):
                nc.tensor.matmul(warm_ps[:], lhsT=warm_x[:, 0:H],
                                 rhs=warm_x[:], start=True, stop=True)

            def epilogue(t, ps1):
                # h = relu(ps1 + b1), fused on DVE, emitted as fp16
                h_sb = hpool.tile([H, NB], f16)
                nc.vector.tensor_scalar(
                    h_sb[:], ps1[:], b1_sb[:], 0.0,
                    mybir.AluOpType.add, mybir.AluOpType.max)
                # out.T[10, NB] = w2 @ h.T
                ps2 = ps2pool.tile([OUT, NB], f32)
                nc.tensor.matmul(ps2[:], lhsT=w2_sb[:], rhs=h_sb[:],
                                 start=True, stop=True)
                # +b2 also on DVE; the store trigger is emitted after the
                # loop so it can never block an x trigger in the ring FIFO
                o_sb = opool.tile([OUT, NB], f32)
                nc.vector.tensor_scalar_add(o_sb[:], ps2[:], b2_sb[:])
                o_tiles.append((t, o_sb))

            o_tiles = []    # (t, o_sb) pending stores, all emitted post-loop
            pending = None  # software pipeline: tile t's epilogue is emitted
                            # after tile t+1's mm1 block so PE never waits on
                            # the DVE relu chain

            for g in range(NT // G):
                x_sb = xpool.tile([KT, G, NK, NB], f16)
                # alternate rings so each ring's per-DMA fixed cost is hidden
                # behind the other ring's transfer (ScalarE runs no compute,
                # so ring-10 triggers issue immediately)
                dma_eng = (nc.sync, nc.scalar)[g % 2]
                dma_eng.dma_start(x_sb[:], xtp[:, g * G:(g + 1) * G, :, :])

                for s in range(G):
                    t = g * G + s
                    # h.T[128, NB] = W_eff.T @ x.T, accumulated over K-tiles.
                    ps1 = ps1pool.tile([H, NB], f32)
                    for k in range(NK):
                        nc.tensor.matmul(
                            ps1[:],
                            lhsT=w_sb[:, k, :],
                            rhs=x_sb[:, s, k, :],
                            start=(k == 0),
                            stop=False,
                        )
                    nc.tensor.matmul(
                        ps1[:], lhsT=wr_sb[:],
                        rhs=xr_sb[:, t * NB:(t + 1) * NB],
                        start=False, stop=True,
                    )
                    if pending is not None:
                        epilogue(*pending)
                    pending = (t, ps1)
            epilogue(*pending)

            # out stores last in the scalar ring's FIFO — after every x
            # trigger — so a store waiting on compute can't stall the stream
            for t, o_sb in o_tiles:
                nc.scalar.dma_start(out[:, t * NB:(t + 1) * NB], o_sb[:])

    nc.compile()
    return nc


def _get_nc():
    if "nc" not in _CACHE:
        _CACHE["nc"] = _build_nc()
    return _CACHE["nc"]


def _fold_weights(conv_w: np.ndarray, w1: np.ndarray) -> np.ndarray:
    """W_eff[784, 128]: h_pre = x @ W_eff  ==  conv(x) @ w1.T  (float64 accum)."""
    w1k = w1.reshape(H, 26, 26).transpose(1, 2, 0).astype(np.float64)  # [i,j,k]
    cw = conv_w.astype(np.float64)
    W = np.zeros((28, 28, H), np.float64)
    for di in range(3):
        for dj in range(3):
            W[di:di + 26, dj:dj + 26, :] += cw[di, dj] * w1k
    return W.reshape(D, H).astype(np.float32)


def make_in_maps(x, conv_w, w1, b1, w2, b2):
    x = np.asarray(x, np.float32)
    weff = np.ascontiguousarray(_fold_weights(
        np.asarray(conv_w, np.float32), np.asarray(w1, np.float32))).astype(np.float16)
    w2t = np.ascontiguousarray(np.asarray(w2, np.float32).T).astype(np.float16)
    b1c = np.ascontiguousarray(np.asarray(b1, np.float32).reshape(H, 1))
    b2c = np.ascontiguousarray(np.asarray(b2, np.float32).reshape(OUT, 1))
    in_maps = []
    for i in range(N_CORES):
        xs = x[i * B_SH:(i + 1) * B_SH].astype(np.float16)  # [8192, 784]
        # main: [t*NB+c, k*KT+p] -> [p, t, k, c]
        xtp = xs[:, :DM].reshape(NT, NB, NK, KT).transpose(3, 0, 2, 1)
        xrem = xs[:, DM:].T                                 # [16, 8192]
        in_maps.append({"xtp": np.ascontiguousarray(xtp),
                        "xrem": np.ascontiguousarray(xrem),
                        "weff": weff, "w2t": w2t, "b1c": b1c, "b2c": b2c})
    return in_maps


def kernel(x, conv_w, w1, b1, w2, b2):
    nc = _get_nc()
    in_maps = make_in_maps(x, conv_w, w1, b1, w2, b2)
    res = run_bass_kernel_spmd(nc, in_maps, list(range(N_CORES)))
    out = np.concatenate([res.results[i]["out"] for i in range(N_CORES)], axis=1)
    return np.ascontiguousarray(out.T)  # [65536, 10] float32


# revision 33
# speedup vs baseline: 1.0054x; 1.0054x over previous
"""Trainium2 Bass kernel for DigitConvolutionalModel.

Math: the 3x3 valid conv on the 28x28 image is a linear map, so it folds into
the first Linear layer:
    out = relu(x @ W_eff + b1) @ w2.T + b2
where W_eff[784, 128] = C @ w1.T and C[784, 676] is the conv-as-matrix built
from conv_w.  W_eff is built on the host (O(1) w.r.t. batch); the device does
the two batch matmuls.

Distribution: pure data parallel — batch dim of x sharded across 8 NeuronCores,
weights replicated.  Each core computes out.T [10, 8192]; the host reassembles
[65536, 10].

Layout: the contraction dim (784 features) is split 6x128 + 16.  The main
768 features ship partition-major as [128, 16, 6, 512] (partition p, batch
tile t, k-tile k, column c; feature f = k*128 + p) — 128-partition DMAs run at
~430 GB/s vs ~270 GB/s for 112-partition ones (unbalanced SDMA engine
assignment), and this is a DMA-roofline kernel.  The 16 remainder features
ship once as xrem [16, 8192] and contribute a K=16 accumulation matmul.

dtypes: x and W_eff ship as fp16 (10 mantissa bits — comparable precision to
the PE's TF32-like fp32r path at 11 bits) — halves HBM traffic and fp16
matmuls run at the full 1 cycle/row PE rate.  Accumulation is fp32 in PSUM;
the hidden activation h = relu(psum + b1) is computed on the DVE and emitted
as fp16 for the second matmul; +b2 rides the ScalarE (Identity activation).
"""

import numpy as np

import concourse.bass as bass  # noqa: F401  (bass registers mybir lowerings)
import concourse.mybir as mybir
import concourse.tile as tile
from concourse import bacc
from concourse.bass_utils import run_bass_kernel_spmd

N_CORES = 8
B = 65536
B_SH = B // N_CORES  # 8192 rows per core
D = 784              # 28*28 input features
DP = 896             # features zero-padded to 7x128 (W pad rows are zero, so
                     # the pad region of x never affects the result)
H = 128              # hidden
OUT = 10
KT = 128             # contraction tile = full partition dim
NK = DP // KT        # 7 K-tiles, all full-partition
NB = 512             # batch columns per tile (= one fp32 PSUM bank)
NT = B_SH // NB      # 16 batch tiles
G = 2                # batch tiles per x DMA (1.5 MB, 12KB/partition runs)

_CACHE = {}


def _build_nc():
    f32 = mybir.dt.float32
    f16 = mybir.dt.float16
    nc = bacc.Bacc("TRN2", target_bir_lowering=False, debug=False,
                   num_devices=N_CORES)
    # x, partition-major: [p, t, k, c] with feature f = k*128 + p (padded)
    xtp = nc.dram_tensor("xtp", [KT, NT, NK, NB], f16,
                         kind="ExternalInput").ap()
    weff = nc.dram_tensor("weff", [DP, H], f16, kind="ExternalInput").ap()
    w2t = nc.dram_tensor("w2t", [H, OUT], f16, kind="ExternalInput").ap()
    b1c = nc.dram_tensor("b1c", [H, 1], f32, kind="ExternalInput").ap()
    b2c = nc.dram_tensor("b2c", [OUT, 1], f32, kind="ExternalInput").ap()
    out = nc.dram_tensor("out", [OUT, B_SH], f32, kind="ExternalOutput").ap()

    with tile.TileContext(nc) as tc:
        with (
            tc.tile_pool(name="wpool", bufs=1) as wpool,
            tc.tile_pool(name="xpool", bufs=6) as xpool,
            tc.tile_pool(name="hpool", bufs=4) as hpool,
            tc.tile_pool(name="opool", bufs=16) as opool,
            tc.tile_pool(name="ps1", bufs=5, space="PSUM") as ps1pool,
            tc.tile_pool(name="ps2", bufs=3, space="PSUM") as ps2pool,
        ):
            # Params + remainder features ride ring 10 (scalar) while the
            # first x group starts immediately on ring 1 (sync).  PE pre-warm:
            # dummy matmuls on a zeroed tile trip the HAM activity monitor to
            # full clock before real data arrives.
            w_sb = wpool.tile([KT, NK, H], f16)
            nc.scalar.dma_start(
                w_sb[:], weff.rearrange("(k p) m -> p k m", p=KT))
            w2_sb = wpool.tile([H, OUT], f16)
            nc.scalar.dma_start(w2_sb[:], w2t[:])
            b1_sb = wpool.tile([H, 1], f32)
            nc.scalar.dma_start(b1_sb[:], b1c[:])
            b2_sb = wpool.tile([OUT, 1], f32)
            nc.scalar.dma_start(b2_sb[:], b2c[:])

            warm_x = wpool.tile([KT, NB], f16)
            nc.vector.memset(warm_x[:], 0.0)
            warm_ps = ps1pool.tile([H, NB], f32, tag="ps1")
            for _ in range(20):
                nc.tensor.matmul(warm_ps[:], lhsT=warm_x[:, 0:H],
                                 rhs=warm_x[:], start=True, stop=True)

            def epilogue(t, ps1):
                # h = relu(ps1 + b1), fused on DVE, emitted as fp16
                h_sb = hpool.tile([H, NB], f16)
                nc.vector.tensor_scalar(
                    h_sb[:], ps1[:], b1_sb[:], 0.0,
                    mybir.AluOpType.add, mybir.AluOpType.max)
                # out.T[10, NB] = w2 @ h.T
                ps2 = ps2pool.tile([OUT, NB], f32)
                nc.tensor.matmul(ps2[:], lhsT=w2_sb[:], rhs=h_sb[:],
                                 start=True, stop=True)
                # +b2 also on DVE; the store trigger is emitted after the
                # loop so it can never block an x trigger in the ring FIFO
                o_sb = opool.tile([OUT, NB], f32)
                nc.vector.tensor_scalar_add(o_sb[:], ps2[:], b2_sb[:])
                o_tiles.append((t, o_sb))

            o_tiles = []    # (t, o_sb) pending stores, all emitted post-loop
            pending = None  # software pipeline: tile t's epilogue is emitted
                            # after tile t+1's mm1 block so PE never waits on
                            # the DVE relu chain

            for g in range(NT // G):
                x_sb = xpool.tile([KT, G, NK, NB], f16)
                # alternate rings so each ring's per-DMA fixed cost is hidden
                # behind the other ring's transfer (ScalarE runs no compute,
                # so ring-10 triggers issue immediately)
                dma_eng = (nc.sync, nc.scalar)[g % 2]
                dma_eng.dma_start(x_sb[:], xtp[:, g * G:(g + 1) * G, :, :])

                for s in range(G):
                    t = g * G + s
                    # h.T[128, NB] = W_eff.T @ x.T, accumulated over K-tiles.
                    ps1 = ps1pool.tile([H, NB], f32)
                    for k in range(NK):
                        nc.tensor.matmul(
                            ps1[:],
                            lhsT=w_sb[:, k, :],
                            rhs=x_sb[:, s, k, :],
                            start=(k == 0),
                            stop=(k == NK - 1),
                        )
                    if pending is not None:
                        epilogue(*pending)
                    pending = (t, ps1)
            epilogue(*pending)

            # out stores last in the scalar ring's FIFO — after every x
            # trigger — so a store waiting on compute can't stall the stream
            for t, o_sb in o_tiles:
                nc.scalar.dma_start(out[:, t * NB:(t + 1) * NB], o_sb[:])

    nc.compile()
    return nc


def _get_nc():
    if "nc" not in _CACHE:
        _CACHE["nc"] = _build_nc()
    return _CACHE["nc"]


def _fold_weights(conv_w: np.ndarray, w1: np.ndarray) -> np.ndarray:
    """W_eff[784, 128]: h_pre = x @ W_eff  ==  conv(x) @ w1.T  (float64 accum)."""
    w1k = w1.reshape(H, 26, 26).transpose(1, 2, 0).astype(np.float64)  # [i,j,k]
    cw = conv_w.astype(np.float64)
    W = np.zeros((28, 28, H), np.float64)
    for di in range(3):
        for dj in range(3):
            W[di:di + 26, dj:dj + 26, :] += cw[di, dj] * w1k
    return W.reshape(D, H).astype(np.float32)


def make_in_maps(x, conv_w, w1, b1, w2, b2):
    x = np.asarray(x, np.float32)
    weff = np.zeros((DP, H), np.float16)
    weff[:D] = _fold_weights(
        np.asarray(conv_w, np.float32), np.asarray(w1, np.float32)).astype(np.float16)
    w2t = np.ascontiguousarray(np.asarray(w2, np.float32).T).astype(np.float16)
    b1c = np.ascontiguousarray(np.asarray(b1, np.float32).reshape(H, 1))
    b2c = np.ascontiguousarray(np.asarray(b2, np.float32).reshape(OUT, 1))
    in_maps = []
    for i in range(N_CORES):
        xs = np.zeros((B_SH, DP), np.float16)               # padded [8192, 896]
        xs[:, :D] = x[i * B_SH:(i + 1) * B_SH]
        # [t*NB+c, k*KT+p] -> [p, t, k, c]
        xtp = xs.reshape(NT, NB, NK, KT).transpose(3, 0, 2, 1)
        in_maps.append({"xtp": np.ascontiguousarray(xtp),
                        "weff": weff, "w2t": w2t, "b1c": b1c, "b2c": b2c})
    return in_maps


def kernel(x, conv_w, w1, b1, w2, b2):
    nc = _get_nc()
    in_maps = make_in_maps(x, conv_w, w1, b1, w2, b2)
    res = run_bass_kernel_spmd(nc, in_maps, list(range(N_CORES)))
    out = np.concatenate([res.results[i]["out"] for i in range(N_CORES)], axis=1)
    return np.ascontiguousarray(out.T)  # [65536, 10] float32
